# revision 1
# baseline (speedup 1.0000x reference)
"""Trainium2 Bass kernel for nn_ContConv1dSim (continuous conv via per-pair kernel MLP).

Sharding: pure data-parallel — batch dim (8) across 8 NeuronCores, params replicated.

Math per batch element b (K=5 lags, Lexp=1276 expanded positions, cin=cout=32, hid=64):
    delta[j,l]  = times[l] - true_times[l//rep + j - K]      (0 outside mask)
    pcf[j,l,:]  = true_features[l//rep + j - K, :]           (0 outside mask)
    te[j,l,c]   = sin(delta[j,l] * freq[c] + phase[c])       (phase=pi/2 on odd c -> cos)
    h[j,l,:]    = relu(te[j,l,:] @ W1 + b1)
    kv[j,l,:,:] = (h[j,l,:] @ W2 + b2).reshape(cin, cout)
    out[l,o]    = sum_{j,i} pcf[j,l,i] * kv[j,l,i,o]

The temporal encoding is computed via the angle-addition identity
    sin(F·t - F·p + ph) = sin(F·t+ph)·cos(F·p) - cos(F·t+ph)·sin(F·p)
with the tiny sin/cos factor tables built on host (ScalarE's Sin LUT only
accepts [-pi, pi], and delta*freq spans ~[-100, 100]).  Device work:
  DVE: te from the 3-term product identity; kv*pcf (broadcast AP) + reduce
  PE : te@W1, h@W2 (per 128-position tile, kv lives only in PSUM)
  ACT: relu(+b1)

Positions are host-padded to LPAD (multiple of 512) so all tiles are uniform
and each input arrives in a single DMA.
"""

import numpy as np

BS, L, K, CIN, COUT, HID = 8, 256, 5, 32, 32, 64
QP = 4  # temporal-encoding quarter-packing factor (128 = QP * CIN partitions)

_CACHE: dict = {}


def _pad_to(LEXP):
    return ((LEXP + 511) // 512) * 512


def _build_program(LEXP: int, repeats: int = 1):
    from contextlib import ExitStack

    import concourse.bacc as bacc
    import concourse.mybir as mybir
    import concourse.tile as tile

    nc = bacc.Bacc("TRN2", target_bir_lowering=False, debug=False)
    dt = mybir.dt.float32

    LPAD = _pad_to(LEXP)
    LQ = LPAD // QP
    P = 128
    n_tiles = LPAD // P
    KVW = CIN * COUT  # 1024

    # Host-prearranged external inputs (each load one contiguous DMA):
    #   arg [p, j*LQ+lq] = wrap(freq_c*delta[j,l] + phase_c), p = q*CIN+c,
    #       l = q*LQ+lq, wrapped to [-pi, pi] (ScalarE Sin LUT range)
    #   pcf [p, ((j*n_tiles+lt)*CIN)+i] = pcf[j, lt*128+p, i]
    #   w1r = tile(W1, (4,1)); w2r = tile(W2, (2,1)); b1r = tile(b1, 2)
    d_arg = nc.dram_tensor("arg", [QP * CIN, K * LQ], dt, kind="ExternalInput").ap()
    d_pcf = nc.dram_tensor(
        "pcf", [P, K * n_tiles * CIN], dt, kind="ExternalInput"
    ).ap()
    d_w1 = nc.dram_tensor("w1r", [P, HID], dt, kind="ExternalInput").ap()
    d_w2 = nc.dram_tensor("w2r", [P, KVW], dt, kind="ExternalInput").ap()
    d_b1 = nc.dram_tensor("b1r", [P, 1], dt, kind="ExternalInput").ap()
    # output staged as [p, lt*COUT+o] = out[lt*128+p, o]; host unscrambles
    d_out = nc.dram_tensor("out", [P, n_tiles * COUT], dt, kind="ExternalOutput").ap()

    PAIRS = [(0, 1), (2, 3)]  # j=4 handled as a half pair

    with tile.TileContext(nc) as tc:
      # repeats>1 replays the whole computation in one NEFF (for slope timing);
      # per-rep ExitStack closes the pools so SBUF/PSUM space is reused.
      for _rep in range(repeats):
       with ExitStack() as ctx:
        consts = ctx.enter_context(tc.tile_pool(name="consts", bufs=1))
        w1t = consts.tile([P, HID], dt, tag="w1")
        nc.sync.dma_start(w1t[:], d_w1[:])
        w2t = consts.tile([P, KVW], dt, tag="w2")
        nc.sync.dma_start(w2t[:], d_w2[:])
        b1t = consts.tile([P, 1], dt, tag="b1")
        nc.sync.dma_start(b1t[:], d_b1[:])
        pcfall = consts.tile([P, K * n_tiles * CIN], dt, tag="pcfall")
        nc.sync.dma_start(pcfall[:], d_pcf[:])
        # arg split per j so stage 1 starts before the whole tensor lands
        argt = consts.tile([QP * CIN, K * LQ], dt, tag="argt")
        for j in range(K):
            nc.sync.dma_start(
                argt[:, j * LQ : (j + 1) * LQ], d_arg[:, j * LQ : (j + 1) * LQ]
            )
        pcfr = pcfall[:].rearrange("p (j t i) -> p j t i", j=K, t=n_tiles, i=CIN)

        # Stage 1 + Stage 2 interleaved per j-pair so the einsum (DVE) starts
        # as soon as the first pair's h is ready.
        # h stored j-paired: hp[0:64] = h_ja, hp[64:128] = h_jb so stage 2 runs
        # two j-streams concurrently in PE row groups.
        hpool = ctx.enter_context(tc.tile_pool(name="h", bufs=2))
        depool = ctx.enter_context(tc.tile_pool(name="de", bufs=2))
        # one PSUM pool: the mm1 scratch tile has the same footprint as a kv2
        # tile, so stage 1 and stage 2 share the two [128, 2048] slots
        kv_pool = ctx.enter_context(tc.tile_pool(name="psum_kv", bufs=2, space="PSUM"))
        tmp_pool = ctx.enter_context(tc.tile_pool(name="tmp", bufs=2))
        red_pool = ctx.enter_context(tc.tile_pool(name="red", bufs=2))
        outb_pool = ctx.enter_context(tc.tile_pool(name="outb", bufs=1))
        outb = outb_pool.tile([P, n_tiles * COUT], dt, tag="outb")

        def emit_te(j):
            teq = depool.tile([QP * CIN, LQ], dt, tag="teq")
            nc.scalar.activation(
                teq[:],
                argt[:, j * LQ : (j + 1) * LQ],
                mybir.ActivationFunctionType.Sin,
            )
            return teq

        def emit_stage1(pi, ja, jb):
            """h for pair (ja, jb) -> hp rows [0:64]=h_ja, [64:128]=h_jb.
            jb None -> half pair (rows 64:128 unwritten/unused)."""
            hp = hpool.tile([P, LPAD], dt, tag="hp", name=f"hp{pi}")
            # mm1 scratch: quarter q lands at bank-aligned column 512*q
            pss = kv_pool.tile([P, 2 * KVW], dt, tag="kv", name=f"ps{pi}")
            halves = ((0, ja),) if jb is None else ((0, ja), (64, jb))
            for half, j in halves:
                teq = emit_te(j)
                for q in range(QP):
                    nc.tensor.matmul(
                        pss[half : half + HID, 512 * q : 512 * q + LQ],
                        w1t[32 * q : 32 * q + 32, :],
                        teq[32 * q : 32 * q + 32, :],
                        start=True,
                        stop=True,
                        tile_position=(32 * q, half),
                    )
            rows = HID if jb is None else P
            for q in range(QP):
                nc.scalar.activation(
                    hp[0:rows, q * LQ : (q + 1) * LQ],
                    pss[0:rows, 512 * q : 512 * q + LQ],
                    mybir.ActivationFunctionType.Relu,
                    bias=b1t[0:rows, :],
                )
            return hp

        # Stage 2 per (pair, l-tile): kv2 = [kv_ja | kv_jb] via two row-group-
        # packed matmul streams; DVE multiplies by pcf (broadcast over cout,
        # pair folded into the op) and reduces (j,i) in one pass.
        for pi, (ja, jb) in enumerate([(0, 1), (2, 3), (4, None)]):
            hp = emit_stage1(pi, ja, jb)
            for lt in range(n_tiles):
                l0 = lt * P
                acc = outb[:, lt * COUT : (lt + 1) * COUT]
                kv2 = kv_pool.tile([P, 2 * KVW], dt, tag="kv")
                halves = ((0, 0),) if jb is None else ((0, 0), (64, KVW))
                for half, joff in halves:
                    for c0 in range(0, KVW, 512):
                        nc.tensor.matmul(
                            kv2[:, joff + c0 : joff + c0 + 512],
                            hp[half : half + HID, l0 : l0 + P],
                            w2t[half : half + HID, c0 : c0 + 512],
                            start=True,
                            stop=True,
                            tile_position=(half, 0),
                        )
                tmp = tmp_pool.tile([P, 2 * KVW], dt, tag="tmp")
                if jb is None:
                    nc.vector.tensor_tensor(
                        tmp[:, 0:KVW],
                        kv2[:, 0:KVW],
                        pcfr[:, ja, lt, :].unsqueeze(2).broadcast_to([P, CIN, COUT]),
                        mybir.AluOpType.mult,
                    )
                    tview = tmp[:, 0:KVW].rearrange(
                        "p (i o) -> p o i", i=CIN, o=COUT
                    )
                    axis = mybir.AxisListType.X
                else:
                    nc.vector.tensor_tensor(
                        tmp[:],
                        kv2[:],
                        pcfr[:, ja : ja + 2, lt, :]
                        .unsqueeze(3)
                        .broadcast_to([P, 2, CIN, COUT]),
                        mybir.AluOpType.mult,
                    )
                    tview = tmp[:].rearrange(
                        "p (j i o) -> p o j i", j=2, i=CIN, o=COUT
                    )
                    axis = mybir.AxisListType.XY
                if pi == 0:
                    nc.vector.tensor_reduce(
                        acc, tview, axis=axis, op=mybir.AluOpType.add
                    )
                else:
                    red = red_pool.tile([P, COUT], dt, tag="red")
                    nc.vector.tensor_reduce(
                        red[:], tview, axis=axis, op=mybir.AluOpType.add
                    )
                    nc.vector.tensor_add(acc, acc, red[:])

        # output: one contiguous DMA of the staged layout
        nc.sync.dma_start(d_out[:], outb[:])

    nc.compile()
    return nc


def _host_prep(times, true_times, true_features, non_pad_mask, sim_size, cin):
    """Index gather/masking + range-wrapped sin arguments (numpy, negligible cost).

    Returns arg (bs, QP*cin, K*LQ) with arg = wrap(freq_c*delta + phase_c) in
    [-pi, pi], and pcf (bs, 128, K*n_tiles*cin) in the staged device layout."""
    bs, Lm = true_times.shape
    LEXP = times.shape[1]
    s = int(sim_size)
    rep = s + 1
    idx = np.arange(Lm)[None, :] + np.arange(K)[:, None]  # (K, L)
    tt_pad = np.pad(true_times.astype(np.float64), ((0, 0), (K, 0)))
    pct = tt_pad[:, idx]  # (bs, K, L)
    tf_pad = np.pad(true_features.astype(np.float32), ((0, 0), (K, 0), (0, 0)))
    pcf = tf_pad[:, idx, :]  # (bs, K, L, cin)
    m_pad = np.pad(non_pad_mask.astype(bool), ((0, 0), (K, 0)))
    dt_mask = m_pad[:, idx] & non_pad_mask[:, None, :].astype(bool)  # (bs, K, L)

    pct = np.repeat(pct, rep, axis=-1)
    pcf = np.repeat(pcf, rep, axis=2)
    dtm = np.repeat(dt_mask, rep, axis=-1)
    if s > 0:
        pct = pct[..., :-s]
        pcf = pcf[:, :, :-s, :]
    dtm = dtm[..., s:]
    assert pct.shape[-1] == LEXP
    # masked slots contribute 0 via pcf=0 (kv stays finite), as in the reference
    pcf = np.where(dtm[..., None], pcf, 0.0).astype(np.float32)

    LPAD = _pad_to(LEXP)
    padl = LPAD - LEXP
    pcf = np.pad(pcf, ((0, 0), (0, 0), (0, padl), (0, 0)))
    delta = times.astype(np.float64)[:, None, :] - pct  # (bs, K, LEXP)
    delta = np.where(dtm, delta, 0.0)
    delta = np.pad(delta, ((0, 0), (0, 0), (0, padl)))

    freq = np.asarray(
        [10000.0 ** (-2.0 * (i // 2) / cin) for i in range(cin)], np.float64
    )
    phase = np.pi / 2.0 * (np.arange(cin) % 2)

    LQ = LPAD // QP
    # arg[b,j,c,l] = freq_c*delta + phase_c, wrapped to [-pi, pi]
    arg = freq[None, None, :, None] * delta[:, :, None, :] + phase[None, None, :, None]
    arg = arg - 2.0 * np.pi * np.round(arg / (2.0 * np.pi))
    arg = np.clip(arg, -np.pi, np.pi)  # guard against rounding just past pi

    # quarter-pack: (bs, K, cin, LPAD) -> (bs, K, QP*cin, LQ) -> (bs, QP*cin, K*LQ)
    aq = arg.reshape(bs, K, cin, QP, LQ)
    aq = np.moveaxis(aq, 3, 2).reshape(bs, K, QP * cin, LQ)
    arg_dev = np.moveaxis(aq, 1, 2).reshape(bs, QP * cin, K * LQ).astype(np.float32)

    # pcf (bs, 128, K*n_tiles*cin): [p, (j, lt, i)] = pcf[j, lt*128+p, i]
    n_tiles = LPAD // 128
    pcfd = (
        pcf.reshape(bs, K, n_tiles, 128, cin)
        .transpose(0, 3, 1, 2, 4)
        .reshape(bs, 128, K * n_tiles * cin)
        .astype(np.float32)
    )
    return arg_dev, pcfd


def _unstage(staged, LEXP):
    # staged [128, n_tiles*COUT] -> [LEXP, COUT]
    P = 128
    n_tiles = staged.shape[1] // COUT
    return (
        staged.reshape(P, n_tiles, COUT)
        .transpose(1, 0, 2)
        .reshape(n_tiles * P, COUT)[:LEXP]
    )


def kernel(times, true_times, true_features, non_pad_mask, W1, b1, W2, b2, sim_size):
    from concourse.bass_utils import run_bass_kernel_spmd

    times = np.asarray(times)
    LEXP = times.shape[1]
    W1 = np.asarray(W1, dtype=np.float32)
    W2 = np.asarray(W2, dtype=np.float32)
    b1 = np.asarray(b1, dtype=np.float32)
    b2 = np.asarray(b2, dtype=np.float32)
    assert np.all(b2 == 0.0), "kernel assumes b2 == 0 (spec fill: zeros)"
    cin = W1.shape[0]

    arg, pcf = _host_prep(
        times, np.asarray(true_times), np.asarray(true_features),
        np.asarray(non_pad_mask), sim_size, cin,
    )

    if LEXP not in _CACHE:
        _CACHE[LEXP] = _build_program(LEXP)
    nc = _CACHE[LEXP]

    in_maps = []
    for b in range(BS):
        in_maps.append(
            {
                "arg": arg[b],
                "pcf": pcf[b],
                "w1r": np.tile(W1, (4, 1)),
                "w2r": np.tile(W2, (2, 1)),
                "b1r": np.tile(b1, 2)[:, None],
            }
        )
    res = run_bass_kernel_spmd(nc, in_maps, core_ids=list(range(BS)))
    out = np.stack([_unstage(res.results[b]["out"], LEXP) for b in range(BS)], axis=0)
    return out.astype(np.float32)



# revision 5
# speedup vs baseline: 5.8360x; 5.8360x over previous
"""Trainium2 Bass kernel for nn_ContConv1dSim (continuous conv via per-pair kernel MLP).

Sharding: pure data-parallel - batch dim (8) across 8 NeuronCores, params replicated.

Key restructure vs the direct form: the post-MLP contraction
    out[l,o] = sum_{j,i} pcf[j,l,i] * kv[j,l,i,o]
uses pcf[j,l,:] = tf_pad[l//5 + j, :] (the sim-grid repeat structure), so with
    V[n][m,o] = sum_i tf_pad[n,i] * W2[m, i*COUT+o]          (261 tiny matrices)
the whole thing becomes
    out[l,o] = sum_j h[j,l,:] . V[l//5 + j][:,o]
i.e. one 64x32 stationary-matmul per n=q+j with a 25-column moving operand
(the 5 l-runs that share V[n]), accumulated over j directly in PSUM.  This
eliminates the (bs,K,Lexp,32,32) elementwise multiply + reduce on DVE that
dominated the direct implementation.

Engine plan per core (LPAD=1280):
  PE : V precompute (32 matmuls), te@W1 (20), stage-2 (~265 small matmuls)
  ACT: sin (temporal encoding), half of relu-bias copies + V copies
  DVE: other half of the copies, PSUM->SBUF output copy
Masking needs no device work: masked slots have tf_pad rows == 0 => V==0.
"""

import numpy as np
import ml_dtypes

BS, L, K, CIN, COUT, HID = 8, 256, 5, 32, 32, 64
QP = 4  # temporal-encoding quarter-packing factor (128 = QP * CIN partitions)
LPAD = 1280
LQ = LPAD // QP  # 320
NQ = L + K  # 261 distinct V matrices
REP = 5

_CACHE: dict = {}

BF16 = ml_dtypes.bfloat16


def _build_program(LEXP: int, repeats: int = 1):
    from contextlib import ExitStack

    import concourse.bacc as bacc
    import concourse.mybir as mybir
    import concourse.tile as tile
    from concourse.ap import AP

    nc = bacc.Bacc("TRN2", target_bir_lowering=False, debug=False)
    f32 = mybir.dt.float32
    bf16 = mybir.dt.bfloat16
    assert LEXP == REP * (L - 1) + 1

    # Host-prearranged external inputs:
    #   arg [p, j*LQ+lq] = wrap(freq_c*delta[j,l] + phase_c), p = qp*CIN+c,
    #       l = qp*LQ+lq, wrapped to [-pi, pi] (ScalarE Sin LUT range)
    #   tft [i, n]           = tf_pad[n, i]                  (masked features^T)
    #   w2t [i, o*HID+m]     = W2[m, i*COUT+o]
    #   w1t [32*qp+i, m]     = W1[i, m]
    #   b1t [m, 1]           = b1[m]
    d_arg = nc.dram_tensor("arg", [QP * CIN, K * LQ], f32, kind="ExternalInput").ap()
    d_tft = nc.dram_tensor("tft", [CIN, NQ], bf16, kind="ExternalInput").ap()
    d_w2t = nc.dram_tensor("w2t", [CIN, COUT * HID], bf16, kind="ExternalInput").ap()
    d_w1t = nc.dram_tensor("w1t", [QP * CIN, HID], bf16, kind="ExternalInput").ap()
    d_b1 = nc.dram_tensor("b1t", [HID, 1], f32, kind="ExternalInput").ap()
    # output staged transposed: out[o, l]; host transposes + trims to LEXP
    d_out = nc.dram_tensor("out", [COUT, LPAD], f32, kind="ExternalOutput").ap()

    HSTR = K * LPAD  # hall row length

    with tile.TileContext(nc) as tc:
      for _rep in range(repeats):
       with ExitStack() as ctx:
        consts = ctx.enter_context(tc.tile_pool(name="consts", bufs=1))
        w2tt = consts.tile([CIN, COUT * HID], bf16, tag="w2t")
        nc.sync.dma_start(w2tt[:], d_w2t[:])
        tftt = consts.tile([CIN, NQ], bf16, tag="tft")
        nc.sync.dma_start(tftt[:], d_tft[:])
        w1tt = consts.tile([QP * CIN, HID], bf16, tag="w1t")
        nc.sync.dma_start(w1tt[:], d_w1t[:])
        b1tt = consts.tile([HID, 1], f32, tag="b1t")
        nc.sync.dma_start(b1tt[:], d_b1[:])
        argt = consts.tile([QP * CIN, K * LQ], f32, tag="argt")
        for j in range(K):
            nc.sync.dma_start(
                argt[:, j * LQ : (j + 1) * LQ], d_arg[:, j * LQ : (j + 1) * LQ]
            )
        teqt = consts.tile([QP * CIN, K * LQ], bf16, tag="teqt")
        hall = consts.tile([HID, K * LPAD], bf16, tag="hall")
        vsb = consts.tile([HID, COUT * NQ], bf16, tag="vsb")  # [m, 32*n+o]
        outb = consts.tile([COUT, LPAD], f32, tag="outb")

        nc.gpsimd.memset(outb[:, 0:REP], 0.0)  # q=0 positions: all-zero V

        # ---- V precompute: V[n][m,o] = sum_i tft[i,n] * w2t[i, o*HID+m] ----
        # per o: out[m, n] = w2t[:, o*HID:(o+1)*HID].T @ tft ; PSUM [64, NQ]
        # 16 groups of 2 o's; copies (cast to bf16) alternate ACT/DVE.
        vsr = vsb[:].rearrange("p (n o) -> p o n", n=NQ, o=COUT)
        with tc.tile_pool(name="vps", bufs=2, space="PSUM") as vpool:
            for og in range(COUT // 2):
                vps = vpool.tile([HID, 1024], f32, tag="vps")
                for oi in range(2):
                    o = 2 * og + oi
                    nc.tensor.matmul(
                        vps[:, 512 * oi : 512 * oi + NQ],
                        w2tt[:, o * HID : (o + 1) * HID],
                        tftt[:, 0:NQ],
                        start=True,
                        stop=True,
                    )
                src = vps[:].rearrange("p (oi n) -> p oi n", oi=2)[:, :, 0:NQ]
                dst = vsr[:, 2 * og : 2 * og + 2, :]
                if og % 2 == 0:
                    nc.vector.tensor_copy(dst, src)
                else:
                    nc.scalar.activation(dst, src, mybir.ActivationFunctionType.Copy)

        # ---- temporal encoding: te = sin(arg) (bf16) ----
        for j in range(K):
            nc.scalar.activation(
                teqt[:, j * LQ : (j + 1) * LQ],
                argt[:, j * LQ : (j + 1) * LQ],
                mybir.ActivationFunctionType.Sin,
            )

        # ---- stage 1: h[j,l,:] = relu(te @ W1 + b1) (bf16) ----
        # psum tile per (j, qp-pair): [64, 2*512]; relu+bias copy to hall.
        hpool = ctx.enter_context(tc.tile_pool(name="hps", bufs=3, space="PSUM"))
        ri = 0
        for pair in range(2):
            for j in range(K):
                hp = hpool.tile([HID, 1024], f32, tag="hp")
                for qq in range(2):
                    qp = 2 * pair + qq
                    nc.tensor.matmul(
                        hp[:, 512 * qq : 512 * qq + LQ],
                        w1tt[32 * qp : 32 * qp + 32, :],
                        teqt[32 * qp : 32 * qp + 32, j * LQ : (j + 1) * LQ],
                        start=True,
                        stop=True,
                        tile_position=(32 * qp, 0),
                    )
                src = hp[:].rearrange("p (qq l) -> p qq l", qq=2)[:, :, 0:LQ]
                dst = (
                    hall[:, j * LPAD + pair * 2 * LQ : j * LPAD + (pair + 1) * 2 * LQ]
                    .rearrange("p (qq l) -> p qq l", qq=2)
                )
                if ri % 2 == 0:
                    nc.scalar.activation(
                        dst, src, mybir.ActivationFunctionType.Relu, bias=b1tt[:]
                    )
                else:
                    nc.vector.tensor_scalar(
                        dst, src, b1tt[:], 0.0,
                        mybir.AluOpType.add, mybir.AluOpType.max,
                    )
                ri += 1

        # ---- stage 2: out[o, l] += sum_j V[l//5+j][:,o] . h[j,l,:] ----
        # one matmul per (n, bank-piece): lhsT = V[n] [64, 32],
        # rhs = h cols l=5q+r for q=n-j (contiguous out cols via j-descending).
        hall_h = hall[:].tensor

        def hofs(n, jp, r):  # flat hall col for (j'=jp, r) of window n
            return 5 * n + 5100 - 1275 * jp + r

        s2pool = ctx.enter_context(tc.tile_pool(name="s2p", bufs=2, space="PSUM"))

        def runs_to_pieces(n, a, b, ws):
            """Split col range [a,b) of window n into AP-able pieces
            (j'-runs; partial runs at the edges)."""
            ka, kb = a - ws, b - ws
            pieces = []
            if ka % 5:
                jp = ka // 5
                r0 = ka % 5
                r1 = min(5, r0 + (kb - ka))
                pieces.append((jp, 1, r0, r1 - r0))
                ka = 5 * (jp + 1)
            if kb // 5 > (ka + 4) // 5:
                jpa = (ka + 4) // 5
                njp = kb // 5 - jpa
                if njp > 0:
                    pieces.append((jpa, njp, 0, 5))
                    ka = 5 * (jpa + njp)
            if ka < kb:
                pieces.append((ka // 5, 1, 0, kb - ka))
            return pieces

        out_chunks = []
        for c in range(3):
            clo, chi = 512 * c, min(512 * (c + 1), LPAD)
            plist = []
            for n in range(K, NQ - 1):
                j0 = max(0, n - (L - 1))
                ws, we = 5 * (n - 4), 5 * (n - j0) + 5
                # "old" cols [ws, 5n) accumulate onto earlier n's writes;
                # "new" cols [5n, we) are first-touch.  Keeping each matmul
                # uniformly one or the other matches the per-bank has_written
                # semantics (and the simulator's all-or-none check).
                for a0, b0 in ((ws, min(we, 5 * n)), (max(ws, 5 * n), we)):
                    a, b = max(a0, clo), min(b0, chi)
                    if a >= b:
                        continue
                    for jp, njp, r0, rc in runs_to_pieces(n, a, b, ws):
                        ofs = hofs(n, jp, r0)
                        dims = [[HSTR, HID]]
                        if njp > 1:
                            dims.append([-1275, njp])
                        dims.append([1, rc])
                        ocol = ws + 5 * jp + r0 - clo
                        plist.append((n, ofs, dims, ocol, njp * rc))
            out_chunks.append((c, clo, chi, plist))

        opool = None
        for c, clo, chi, plist in out_chunks:
            s2p = s2pool.tile([COUT, 512], f32, tag="s2p")
            s2t = s2p[:].tensor
            for idx, (n, ofs, dims, ocol, ncols) in enumerate(plist):
                rhs = AP(hall_h, ofs, dims)
                outap = AP(s2t, ocol, [[512, COUT], [1, ncols]])
                nc.tensor.matmul(
                    outap,
                    vsb[:, COUT * n : COUT * (n + 1)],
                    rhs,
                    start=(idx == 0),
                    stop=(idx == len(plist) - 1),
                    skip_group_check=True,
                    tile_position=(0, 0),
                )
            lo = REP if c == 0 else 0  # cols 0..4 of chunk 0 never written
            nc.vector.tensor_copy(
                outb[:, clo + lo : chi], s2p[:, lo : chi - clo]
            )

        nc.sync.dma_start(d_out[:], outb[:])

    nc.compile()
    return nc


def _host_prep(times, true_times, true_features, non_pad_mask, sim_size, cin):
    """Gather/mask + range-wrapped sin arguments and staged tensors (numpy)."""
    bs, Lm = true_times.shape
    LEXP = times.shape[1]
    s = int(sim_size)
    rep = s + 1
    assert rep == REP and Lm == L and cin == CIN

    tt_pad = np.pad(true_times.astype(np.float64), ((0, 0), (K, 0)))
    mask = non_pad_mask.astype(bool)
    tf_pad = np.pad(
        true_features.astype(np.float32) * mask[:, :, None],
        ((0, 0), (K, 0), (0, 0)),
    )  # (bs, NQ, CIN)

    q = np.arange(LEXP) // rep
    # delta[b, j, l] = times[b, l] - tt_pad[b, q+j]
    delta = (
        times.astype(np.float64)[:, None, :]
        - np.stack([tt_pad[:, q + j] for j in range(K)], axis=1)
    )  # (bs, K, LEXP)
    padl = LPAD - LEXP
    delta = np.pad(delta, ((0, 0), (0, 0), (0, padl)))

    freq = np.asarray(
        [10000.0 ** (-2.0 * (i // 2) / cin) for i in range(cin)], np.float64
    )
    phase = np.pi / 2.0 * (np.arange(cin) % 2)
    arg = freq[None, None, :, None] * delta[:, :, None, :] + phase[None, None, :, None]
    arg = arg - 2.0 * np.pi * np.round(arg / (2.0 * np.pi))
    arg = np.clip(arg, -np.pi, np.pi)

    # quarter-pack: (bs, K, cin, LPAD) -> (bs, QP*cin, K*LQ)
    aq = arg.reshape(bs, K, cin, QP, LQ)
    aq = np.moveaxis(aq, 3, 2).reshape(bs, K, QP * cin, LQ)
    arg_dev = np.moveaxis(aq, 1, 2).reshape(bs, QP * cin, K * LQ).astype(np.float32)

    tft = np.ascontiguousarray(tf_pad.transpose(0, 2, 1)).astype(BF16)  # (bs,32,261)
    return arg_dev, tft


def kernel(times, true_times, true_features, non_pad_mask, W1, b1, W2, b2, sim_size):
    from concourse.bass_utils import run_bass_kernel_spmd

    times = np.asarray(times)
    LEXP = times.shape[1]
    W1 = np.asarray(W1, dtype=np.float32)
    W2 = np.asarray(W2, dtype=np.float32)
    b1 = np.asarray(b1, dtype=np.float32)
    b2 = np.asarray(b2, dtype=np.float32)
    assert np.all(b2 == 0.0), "kernel assumes b2 == 0 (spec fill: zeros)"
    cin = W1.shape[0]

    arg, tft = _host_prep(
        times, np.asarray(true_times), np.asarray(true_features),
        np.asarray(non_pad_mask), sim_size, cin,
    )
    # w2t[i, o*HID+m] = W2[m, i*COUT+o]
    w2t = np.ascontiguousarray(
        W2.reshape(HID, CIN, COUT).transpose(1, 2, 0)
    ).reshape(CIN, COUT * HID).astype(BF16)
    w1t = np.tile(W1, (QP, 1)).astype(BF16)
    b1t = b1[:, None].astype(np.float32)

    if LEXP not in _CACHE:
        _CACHE[LEXP] = _build_program(LEXP)
    nc = _CACHE[LEXP]

    in_maps = [
        {"arg": arg[b], "tft": tft[b], "w2t": w2t, "w1t": w1t, "b1t": b1t}
        for b in range(BS)
    ]
    res = run_bass_kernel_spmd(nc, in_maps, core_ids=list(range(BS)))
    out = np.stack(
        [res.results[b]["out"][:, :LEXP].T for b in range(BS)], axis=0
    )
    return out.astype(np.float32)


# revision 14
# speedup vs baseline: 10.3331x; 1.7706x over previous
"""Trainium2 Bass kernel for nn_ContConv1dSim (continuous conv via per-pair kernel MLP).

Sharding: pure data-parallel - batch dim (8) across 8 NeuronCores, params replicated.

Key restructure vs the direct form: the post-MLP contraction
    out[l,o] = sum_{j,i} pcf[j,l,i] * kv[j,l,i,o]
uses pcf[j,l,:] = tf_pad[l//5 + j, :] (the sim-grid repeat structure), so with
    V[n][m,o] = sum_i tf_pad[n,i] * W2[m, i*COUT+o]          (261 tiny matrices)
the whole thing becomes
    out[l,o] = sum_j h[j,l,:] . V[l//5 + j][:,o]
i.e. one 64x32 stationary-matmul per n=q+j with a 25-column moving operand
(the 5 l-runs that share V[n]), accumulated over j directly in PSUM.  This
eliminates the (bs,K,Lexp,32,32) elementwise multiply + reduce on DVE that
dominated the direct implementation.

Engine plan per core (LPAD=1280):
  PE : V precompute (32 matmuls), te@W1 (20), stage-2 (~265 small matmuls)
  ACT: sin (temporal encoding), half of relu-bias copies + V copies
  DVE: other half of the copies, PSUM->SBUF output copy
Masking needs no device work: masked slots have tf_pad rows == 0 => V==0.
"""

import numpy as np
import ml_dtypes

BS, L, K, CIN, COUT, HID = 8, 256, 5, 32, 32, 64
QP = 4  # temporal-encoding quarter-packing factor (128 = QP * CIN partitions)
LPAD = 1280
LQ = LPAD // QP  # 320
NQ = L + K  # 261 distinct V matrices
REP = 5

_CACHE: dict = {}

BF16 = ml_dtypes.bfloat16


def _build_program(LEXP: int, repeats: int = 1):
    from contextlib import ExitStack

    import concourse.bacc as bacc
    import concourse.mybir as mybir
    import concourse.tile as tile
    from concourse.ap import AP

    nc = bacc.Bacc("TRN2", target_bir_lowering=False, debug=False)
    f32 = mybir.dt.float32
    bf16 = mybir.dt.bfloat16
    assert LEXP == REP * (L - 1) + 1

    # Host-prearranged external inputs:
    #   arg [p, j*LQ+lq] = wrap(freq_c*delta[j,l] + phase_c), p = qp*CIN+c,
    #       l = qp*LQ+lq, wrapped to [-pi, pi] (ScalarE Sin LUT range)
    #   tft [i, n]           = tf_pad[n, i]                  (masked features^T)
    #   w2t [i, o*HID+m]     = W2[m, i*COUT+o]
    #   w1t [32*qp+i, m]     = W1[i, m]
    #   b1t [m, 1]           = b1[m]
    d_arg = nc.dram_tensor("arg", [QP * CIN, K * LQ], f32, kind="ExternalInput").ap()
    d_tft = nc.dram_tensor("tft", [CIN, NQ], bf16, kind="ExternalInput").ap()
    d_w2t = nc.dram_tensor("w2t", [CIN, COUT * HID], bf16, kind="ExternalInput").ap()
    d_w1t = nc.dram_tensor("w1t", [QP * CIN, HID], bf16, kind="ExternalInput").ap()
    d_b1 = nc.dram_tensor("b1t", [HID, 1], f32, kind="ExternalInput").ap()
    # output staged transposed: out[o, l]; host transposes + trims to LEXP
    d_out = nc.dram_tensor("out", [COUT, LPAD], f32, kind="ExternalOutput").ap()

    HSTR = K * LPAD  # hall row length

    with tile.TileContext(nc) as tc:
      for _rep in range(repeats):
       with ExitStack() as ctx:
        consts = ctx.enter_context(tc.tile_pool(name="consts", bufs=1))
        tftt = consts.tile([CIN, NQ], bf16, tag="tft")
        nc.gpsimd.dma_start(tftt[:], d_tft[:])
        w2tt = consts.tile([CIN, COUT * HID], bf16, tag="w2t")
        for ch in range(4):
            c0 = ch * (COUT * HID // 4)
            c1 = c0 + COUT * HID // 4
            nc.sync.dma_start(w2tt[:, c0:c1], d_w2t[:, c0:c1])
        w1tt = consts.tile([QP * CIN, HID], bf16, tag="w1t")
        b1tt = consts.tile([HID, 1], f32, tag="b1t")
        argt = consts.tile([QP * CIN, K * LQ], f32, tag="argt")
        # issue the remaining input DMAs from otherwise-idle queues so they
        # don't serialize behind w2t/tft on SP
        for j in range(K):
            nc.gpsimd.dma_start(
                argt[:, j * LQ : (j + 1) * LQ], d_arg[:, j * LQ : (j + 1) * LQ]
            )
        nc.gpsimd.dma_start(w1tt[:], d_w1t[:])
        nc.gpsimd.dma_start(b1tt[:], d_b1[:])
        teqt = consts.tile([QP * CIN, K * LQ], bf16, tag="teqt")
        hall = consts.tile([HID, K * LPAD], bf16, tag="hall")
        vsb = consts.tile([HID, COUT * NQ], bf16, tag="vsb")  # [m, 32*n+o]
        outb = consts.tile([COUT, LPAD], f32, tag="outb")

        nc.gpsimd.memset(outb[:, 0:REP], 0.0)  # q=0 positions: all-zero V

        # ---- V precompute: V[n][m,o] = sum_i tft[i,n] * w2t[i, o*HID+m] ----
        # per o: out[m, n] = w2t[:, o*HID:(o+1)*HID].T @ tft ; PSUM [64, NQ]
        # 16 groups of 2 o's; copies (cast to bf16) alternate ACT/DVE.
        vsr = vsb[:].rearrange("p (n o) -> p o n", n=NQ, o=COUT)
        hpool = ctx.enter_context(tc.tile_pool(name="hps", bufs=2, space="PSUM"))
        vpool_cm = tc.tile_pool(name="vps", bufs=2, space="PSUM")
        vpool = vpool_cm.__enter__()

        def emit_vgroup(og):
            # V[n][m,o] for o in {2og, 2og+1}: out[m,n] = W2_o.T @ tft
            vps = vpool.tile([HID, 1024], f32, tag="vps")
            for oi in range(2):
                o = 2 * og + oi
                nc.tensor.matmul(
                    vps[:, 512 * oi : 512 * oi + NQ],
                    w2tt[:, o * HID : (o + 1) * HID],
                    tftt[:, 0:NQ],
                    start=True,
                    stop=True,
                )
            src = vps[:].rearrange("p (oi n) -> p oi n", oi=2)[:, :, 0:NQ]
            dst = vsr[:, 2 * og : 2 * og + 2, :]
            if og % 2 == 0:
                nc.vector.tensor_copy(dst, src)
            else:
                nc.scalar.activation(dst, src, mybir.ActivationFunctionType.Copy)

        def emit_sin(j):
            nc.scalar.activation(
                teqt[:, j * LQ : (j + 1) * LQ],
                argt[:, j * LQ : (j + 1) * LQ],
                mybir.ActivationFunctionType.Sin,
            )

        def emit_mm1(j, pair, ri):
            hp = hpool.tile([HID, 1024], f32, tag="hp")
            for qq in range(2):
                qp = 2 * pair + qq
                nc.tensor.matmul(
                    hp[:, 512 * qq : 512 * qq + LQ],
                    w1tt[32 * qp : 32 * qp + 32, :],
                    teqt[32 * qp : 32 * qp + 32, j * LQ : (j + 1) * LQ],
                    start=True,
                    stop=True,
                    tile_position=(32 * qp, 0),
                )
            src = hp[:].rearrange("p (qq l) -> p qq l", qq=2)[:, :, 0:LQ]
            dst = (
                hall[:, j * LPAD + pair * 2 * LQ : j * LPAD + (pair + 1) * 2 * LQ]
                .rearrange("p (qq l) -> p qq l", qq=2)
            )
            if ri % 2 == 0:
                nc.scalar.activation(
                    dst, src, mybir.ActivationFunctionType.Relu, bias=b1tt[:]
                )
            else:
                nc.vector.tensor_scalar(
                    dst, src, b1tt[:], 0.0,
                    mybir.AluOpType.add, mybir.AluOpType.max,
                )

        # Interleave V groups with sin/mm1 units so PE fills V-copy waits
        # with stage-1 work (and vice versa).
        mm1_units = [(j, pair) for pair in range(2) for j in range(K)]
        for j in range(K):
            emit_sin(j)
        vg = 0
        for og in range(6):
            emit_vgroup(vg)
            vg += 1
        for ri, (j, pair) in enumerate(mm1_units):
            if vg < COUT // 2:
                emit_vgroup(vg)
                vg += 1
            emit_mm1(j, pair, ri)
        while vg < COUT // 2:
            emit_vgroup(vg)
            vg += 1
        vpool_cm.__exit__(None, None, None)  # free V psum banks for stage 2

        # ---- stage 2: out[o, l] += sum_j V[l//5+j][:,o] . h[j,l,:] ----
        # one matmul per (n, bank-piece): lhsT = V[n] [64, 32],
        # rhs = h cols l=5q+r for q=n-j (contiguous out cols via j-descending).
        hall_h = hall[:].tensor

        def hofs(n, jp, r):  # flat hall col for (j'=jp, r) of window n
            return 5 * n + 5100 - 1275 * jp + r

        s2pool = ctx.enter_context(tc.tile_pool(name="s2p", bufs=2, space="PSUM"))

        def runs_to_pieces(n, a, b, ws):
            """Split col range [a,b) of window n into AP-able pieces
            (j'-runs; partial runs at the edges)."""
            ka, kb = a - ws, b - ws
            pieces = []
            if ka % 5:
                jp = ka // 5
                r0 = ka % 5
                r1 = min(5, r0 + (kb - ka))
                pieces.append((jp, 1, r0, r1 - r0))
                ka = 5 * (jp + 1)
            if kb // 5 > (ka + 4) // 5:
                jpa = (ka + 4) // 5
                njp = kb // 5 - jpa
                if njp > 0:
                    pieces.append((jpa, njp, 0, 5))
                    ka = 5 * (jpa + njp)
            if ka < kb:
                pieces.append((ka // 5, 1, 0, kb - ka))
            return pieces

        out_chunks = []
        for c in range(3):
            clo, chi = 512 * c, min(512 * (c + 1), LPAD)
            plist = []
            for n in range(K, NQ - 1):
                j0 = max(0, n - (L - 1))
                ws, we = 5 * (n - 4), 5 * (n - j0) + 5
                # "old" cols [ws, 5n) accumulate onto earlier n's writes;
                # "new" cols [5n, we) are first-touch.  Keeping each matmul
                # uniformly one or the other matches the per-bank has_written
                # semantics (and the simulator's all-or-none check).
                for a0, b0 in ((ws, min(we, 5 * n)), (max(ws, 5 * n), we)):
                    a, b = max(a0, clo), min(b0, chi)
                    if a >= b:
                        continue
                    for jp, njp, r0, rc in runs_to_pieces(n, a, b, ws):
                        ofs = hofs(n, jp, r0)
                        dims = [[HSTR, HID]]
                        if njp > 1:
                            dims.append([-1275, njp])
                        dims.append([1, rc])
                        ocol = ws + 5 * jp + r0 - clo
                        plist.append((n, ofs, dims, ocol, njp * rc))
            out_chunks.append((c, clo, chi, plist))

        opool = None
        for c, clo, chi, plist in out_chunks:
            s2p = s2pool.tile([COUT, 512], f32, tag="s2p")
            s2t = s2p[:].tensor
            for idx, (n, ofs, dims, ocol, ncols) in enumerate(plist):
                rhs = AP(hall_h, ofs, dims)
                outap = AP(s2t, ocol, [[512, COUT], [1, ncols]])
                nc.tensor.matmul(
                    outap,
                    vsb[:, COUT * n : COUT * (n + 1)],
                    rhs,
                    start=(idx == 0),
                    stop=(idx == len(plist) - 1),
                    skip_group_check=True,
                    tile_position=(0, 0),
                )
            lo = REP if c == 0 else 0  # cols 0..4 of chunk 0 never written
            nc.vector.tensor_copy(
                outb[:, clo + lo : chi], s2p[:, lo : chi - clo]
            )
            # per-chunk output DMA so the tail only pays for the last chunk
            nc.sync.dma_start(d_out[:, clo:chi], outb[:, clo:chi])

    nc.compile()
    return nc


def _host_prep(times, true_times, true_features, non_pad_mask, sim_size, cin):
    """Gather/mask + range-wrapped sin arguments and staged tensors (numpy)."""
    bs, Lm = true_times.shape
    LEXP = times.shape[1]
    s = int(sim_size)
    rep = s + 1
    assert rep == REP and Lm == L and cin == CIN

    tt_pad = np.pad(true_times.astype(np.float64), ((0, 0), (K, 0)))
    mask = non_pad_mask.astype(bool)
    tf_pad = np.pad(
        true_features.astype(np.float32) * mask[:, :, None],
        ((0, 0), (K, 0), (0, 0)),
    )  # (bs, NQ, CIN)

    q = np.arange(LEXP) // rep
    # delta[b, j, l] = times[b, l] - tt_pad[b, q+j]
    delta = (
        times.astype(np.float64)[:, None, :]
        - np.stack([tt_pad[:, q + j] for j in range(K)], axis=1)
    )  # (bs, K, LEXP)
    padl = LPAD - LEXP
    delta = np.pad(delta, ((0, 0), (0, 0), (0, padl)))

    freq = np.asarray(
        [10000.0 ** (-2.0 * (i // 2) / cin) for i in range(cin)], np.float64
    )
    phase = np.pi / 2.0 * (np.arange(cin) % 2)
    arg = freq[None, None, :, None] * delta[:, :, None, :] + phase[None, None, :, None]
    arg = arg - 2.0 * np.pi * np.round(arg / (2.0 * np.pi))
    arg = np.clip(arg, -np.pi, np.pi)

    # quarter-pack: (bs, K, cin, LPAD) -> (bs, QP*cin, K*LQ)
    aq = arg.reshape(bs, K, cin, QP, LQ)
    aq = np.moveaxis(aq, 3, 2).reshape(bs, K, QP * cin, LQ)
    arg_dev = np.moveaxis(aq, 1, 2).reshape(bs, QP * cin, K * LQ).astype(np.float32)

    tft = np.ascontiguousarray(tf_pad.transpose(0, 2, 1)).astype(BF16)  # (bs,32,261)
    return arg_dev, tft


def kernel(times, true_times, true_features, non_pad_mask, W1, b1, W2, b2, sim_size):
    from concourse.bass_utils import run_bass_kernel_spmd

    times = np.asarray(times)
    LEXP = times.shape[1]
    W1 = np.asarray(W1, dtype=np.float32)
    W2 = np.asarray(W2, dtype=np.float32)
    b1 = np.asarray(b1, dtype=np.float32)
    b2 = np.asarray(b2, dtype=np.float32)
    assert np.all(b2 == 0.0), "kernel assumes b2 == 0 (spec fill: zeros)"
    cin = W1.shape[0]

    arg, tft = _host_prep(
        times, np.asarray(true_times), np.asarray(true_features),
        np.asarray(non_pad_mask), sim_size, cin,
    )
    # w2t[i, o*HID+m] = W2[m, i*COUT+o]
    w2t = np.ascontiguousarray(
        W2.reshape(HID, CIN, COUT).transpose(1, 2, 0)
    ).reshape(CIN, COUT * HID).astype(BF16)
    w1t = np.tile(W1, (QP, 1)).astype(BF16)
    b1t = b1[:, None].astype(np.float32)

    if LEXP not in _CACHE:
        _CACHE[LEXP] = _build_program(LEXP)
    nc = _CACHE[LEXP]

    in_maps = [
        {"arg": arg[b], "tft": tft[b], "w2t": w2t, "w1t": w1t, "b1t": b1t}
        for b in range(BS)
    ]
    res = run_bass_kernel_spmd(nc, in_maps, core_ids=list(range(BS)))
    out = np.stack(
        [res.results[b]["out"][:, :LEXP].T for b in range(BS)], axis=0
    )
    return out.astype(np.float32)
